# revision 26
# baseline (speedup 1.0000x reference)
"""Trainium2 Bass kernel for the GAT-style message-passing layer.

Math reduction (exact w.r.t. the reference's masking semantics):
  h = x @ W is rank-1, so with c1 = W@a1, c2 = W@a2:
    e[b,i,j] = leakyrelu(c1*x_bi + c2*x_bj)
  After adjacency AND positivity masking, the softmax rows reduce to
    att[b,i,j] = m_ij * w_bj / D_bi,   w_bj = exp(c2*x_bj),
    D_bi = sum_j m_ij*w_bj,            m_ij = (adj_ij>0) & (c1*x_bi+c2*x_bj>0)
  (the exp(c1*x_bi) row factor cancels).  Then
    out[b,i,:] = ELU(s_bi * W),  s_bi = P_bi / D_bi,  P_bi = sum_j m_ij*w_bj*x_bj
  Fully-masked rows (D_bi == 0) fall back to the uniform softmax:
    s_bi = mean_j x_bj.

Sharding (8 cores): 4 row-blocks of the N dimension x 2 batch halves.
Each core owns 512 attention rows for 8 batches; it reads only its 4MB
row-slice of adj (transposed on host so j lands on partitions).

Per core the N^2 work is one fused mask op per [128 j x 512 i] chunk:
    R[j,i] = (c1*x_i  is_gt  -c2*x_j) * adjT[j,i]        (exact 0/1, bf16)
split across DVE (scalar_tensor_tensor) and ACT (saturated Sigmoid step,
with the adjacency multiply on Pool or DVE).  bf16 TensorE matmuls with the
chunk mask as the stationary and [w_j, w_j*x_j] as the moving operand reduce
each chunk into per-row (D, P) PSUM columns.  The epilogue computes
s = P/D with the degenerate-row blend and applies ELU(s*W) elementwise.
"""

import sys

import numpy as np

sys.path.insert(0, "/opt/trn_rl_repo")

import ml_dtypes  # noqa: E402

BS = 16
N = 2048
F = 40
NCORES = 8
NRB = 4                   # row blocks
NBH = 2                   # batch halves
RB = N // NRB             # 512 attention rows per core
BH = BS // NBH            # 8 batches per core
NCHUNK = N // 128         # 16 j-chunks
NH = RB // 128            # 4 stationary halves per chunk
BIGF = 1.0e30             # saturation scale for the ACT step path
ROWS_PER_CORE = BH * RB   # 4096 output rows per core
NK = ROWS_PER_CORE // 128 # 32 output chunks

# mask-chunk engine assignment by j-chunk index c (16 entries):
#   'd'  -> DVE scalar_tensor_tensor (one fused op)
#   'av' -> ACT saturated-Sigmoid step + DVE bf16 tensor_tensor multiply
#   'ap' -> ACT saturated-Sigmoid step + Pool tensor_tensor multiply
IMPL = ["d", "av", "ap", "av", "ap", "d", "av", "ap", "av", "d", "ap", "av", "ap", "d", "av", "ap"]
USE_SIGN_PATH = False     # fallback: exact Sign+Relu ACT step (2 ACT passes)


def _build(c1: float, c2: float):
    import concourse.bass as bass  # noqa: F401
    import concourse.tile as tile
    from concourse import bacc, mybir

    f32 = mybir.dt.float32
    bf16 = mybir.dt.bfloat16
    Alu = mybir.AluOpType
    Act = mybir.ActivationFunctionType

    nc = bacc.Bacc("TRN2", target_bir_lowering=False, debug=False)

    adjT_b = nc.declare_dram_parameter("adjT_b", [N, RB], bf16, isOutput=False)
    xc1b = nc.declare_dram_parameter("xc1b", [BH, RB], f32, isOutput=False)
    xt = nc.declare_dram_parameter("xt", [BH, 128, NCHUNK], f32, isOutput=False)
    xmean = nc.declare_dram_parameter("xmean", [1, NK], f32, isOutput=False)
    wmat = nc.declare_dram_parameter("wmat", [1, F], f32, isOutput=False)
    out_e = nc.declare_dram_parameter("out", [ROWS_PER_CORE, F], f32, isOutput=True)

    with tile.TileContext(nc) as tc:
        with (
            tc.tile_pool(name="const", bufs=1) as const,
            tc.tile_pool(name="wtmp", bufs=2) as wtmp_p,
            tc.tile_pool(name="xrep", bufs=3) as xrep_p,
            tc.tile_pool(name="gt", bufs=4) as gt_p,
            tc.tile_pool(name="rt", bufs=6) as rt_p,
            tc.tile_pool(name="acc", bufs=1, space="PSUM") as acc_p,
            tc.tile_pool(name="dp", bufs=1) as dp_p,
            tc.tile_pool(name="ep", bufs=1) as ep_p,
            tc.tile_pool(name="og", bufs=1) as og_p,
        ):
            # ---- constants / prologue -------------------------------------
            xt_t = const.tile([128, BH * NCHUNK], f32)  # col b*16+c = x[b, chunk c]
            nc.scalar.dma_start(xt_t[:], xt.rearrange("b p c -> p b c"))

            wrep = const.tile([128, F], f32)
            nc.scalar.dma_start(wrep[:], wmat[0:1, :].broadcast_to([128, F]))

            AGRP = 4  # adjacency chunk-group tiles (finer DMA/dep granularity)
            GC = NCHUNK // AGRP
            a_grp = []
            for g in range(AGRP):
                a_g = const.tile([128, GC * RB], bf16, tag=f"ag{g}")
                nc.sync.dma_start(
                    a_g[:],
                    adjT_b[g * GC * 128 : (g + 1) * GC * 128, :].rearrange(
                        "(c p) i -> p c i", p=128
                    ),
                )
                a_grp.append(a_g)

            # nv = -c2*x (stt threshold), bact = BIGF*c2*x (ACT step bias)
            nv_t = const.tile([128, BH * NCHUNK], f32)
            nc.gpsimd.tensor_scalar_mul(nv_t[:], xt_t[:], -c2)
            bact_t = const.tile([128, BH * NCHUNK], f32)
            nc.gpsimd.tensor_scalar_mul(bact_t[:], xt_t[:], BIGF * c2)

            # wy: interleaved [w_j, w_j*x_j] bf16 columns, 2 per (b, chunk)
            wy = const.tile([128, BH * 2 * NCHUNK], bf16)
            last_exp = None
            for b in range(BH):
                xb = xt_t[:, b * NCHUNK : (b + 1) * NCHUNK]
                w_f = wtmp_p.tile([128, NCHUNK], f32, tag="w_f")
                last_exp = nc.scalar.activation(w_f[:], xb, Act.Exp, bias=0.0, scale=c2)
                y_f = wtmp_p.tile([128, NCHUNK], f32, tag="y_f")
                nc.vector.tensor_mul(y_f[:], w_f[:], xb)
                base = b * 2 * NCHUNK
                nc.vector.tensor_copy(wy[:, base : base + 2 * NCHUNK : 2], w_f[:])
                nc.vector.tensor_copy(wy[:, base + 1 : base + 2 * NCHUNK : 2], y_f[:])

            # ---- main loop: masks + (D, P) reductions ---------------------
            # accs[h][:, 2b:2b+2] accumulates (D, P) for stationary half h of
            # batch b; one PSUM bank per half so accumulation groups never
            # share a bank's zero region.
            acc_0 = acc_p.tile([128, 2 * BH], f32, tag="acc0")
            acc_1 = acc_p.tile([128, 2 * BH], f32, tag="acc1")
            acc_2 = acc_p.tile([128, 2 * BH], f32, tag="acc2")
            acc_3 = acc_p.tile([128, 2 * BH], f32, tag="acc3")
            accs = [acc_0, acc_1, acc_2, acc_3]
            for b in range(BH):
                xr = xrep_p.tile([128, RB], f32)
                nc.scalar.dma_start(xr[:], xc1b[b : b + 1, :].broadcast_to([128, RB]))
                for c in range(NCHUNK):
                    col = b * NCHUNK + c
                    a_chunk_b = a_grp[c // 4][:, (c % 4) * RB : (c % 4 + 1) * RB]
                    r = rt_p.tile([128, RB], bf16)
                    kind = IMPL[c]
                    if kind.startswith("a"):
                        g = gt_p.tile([128, RB], bf16)
                        if USE_SIGN_PATH:
                            g2 = gt_p.tile([128, RB], f32, tag="g2")
                            nc.scalar.activation(
                                g2[:], xr[:], Act.Sign,
                                bias=bact_t[:, col : col + 1], scale=BIGF,
                            )
                            nc.scalar.activation(g[:], g2[:], Act.Relu)
                        else:
                            sig = nc.scalar.activation(
                                g[:], xr[:], Act.Sigmoid,
                                bias=bact_t[:, col : col + 1], scale=BIGF,
                            )
                            if last_exp is not None:
                                from concourse.tile import add_dep_helper
                                add_dep_helper(
                                    sig.ins, last_exp.ins,
                                    reason="act table: exps before sigmoids",
                                )
                                last_exp = None
                        eng = nc.vector if kind == "av" else nc.gpsimd
                        eng.tensor_mul(r[:], g[:], a_chunk_b)
                    else:
                        nc.vector.scalar_tensor_tensor(
                            r[:], xr[:], nv_t[:, col : col + 1], a_chunk_b,
                            Alu.is_gt, Alu.mult,
                        )
                    wy_c = wy[:, b * 2 * NCHUNK + 2 * c : b * 2 * NCHUNK + 2 * c + 2]
                    for h in range(NH):
                        nc.tensor.matmul(
                            accs[h][:, 2 * b : 2 * b + 2],
                            r[:, h * 128 : (h + 1) * 128],
                            wy_c,
                            start=(c == 0), stop=(c == NCHUNK - 1),
                        )

            # ---- epilogue: s = P/D with uniform fallback ------------------
            # epilogue column t = h*BH + b (half-major)
            dp_sb = dp_p.tile([128, 2 * NK], f32)
            for h in range(NH):
                nc.vector.tensor_copy(
                    dp_sb[:, h * 2 * BH : (h + 1) * 2 * BH], accs[h][:]
                )
            d_v = dp_sb[:, 0 : 2 * NK : 2]   # [128, 32]
            p_v = dp_sb[:, 1 : 2 * NK : 2]   # [128, 32]
            xmean_r = const.tile([128, NK], f32)
            nc.sync.dma_start(xmean_r[:], xmean[0:1, :].broadcast_to([128, NK]))
            dmax = ep_p.tile([128, NK], f32)
            nc.vector.tensor_scalar_max(dmax[:], d_v, 1e-30)
            rec = ep_p.tile([128, NK], f32)
            nc.vector.reciprocal(rec[:], dmax[:])
            s0 = ep_p.tile([128, NK], f32)
            nc.vector.tensor_mul(s0[:], p_v, rec[:])
            flag = ep_p.tile([128, NK], f32)
            nc.vector.tensor_scalar(flag[:], d_v, 0.0, None, Alu.is_gt)
            t1 = ep_p.tile([128, NK], f32)
            nc.vector.tensor_sub(t1[:], s0[:], xmean_r[:])
            t2 = ep_p.tile([128, NK], f32)
            nc.vector.tensor_mul(t2[:], t1[:], flag[:])
            s = ep_p.tile([128, NK], f32)
            nc.vector.tensor_add(s[:], t2[:], xmean_r[:])

            # ---- output: ELU(s * W); chunk k covers out rows [128k, 128k+128)
            t_all = og_p.tile([128, NK * F], f32, tag="t_all")
            for k in range(NK):
                col = (k % NH) * BH + (k // NH)  # half-major epilogue column
                dst = t_all[:, k * F : (k + 1) * F]
                if k % 2 == 0:
                    nc.vector.tensor_scalar_mul(dst, wrep[:], s[:, col : col + 1])
                else:
                    nc.scalar.activation(
                        dst, wrep[:], Act.Copy, bias=0.0, scale=s[:, col : col + 1]
                    )
            mn = og_p.tile([128, NK * F], f32, tag="mn")
            nc.vector.tensor_scalar_min(mn[:], t_all[:], 0.0)
            rt2 = og_p.tile([128, NK * F], f32, tag="rt2")
            nc.vector.tensor_scalar_max(rt2[:], t_all[:], 0.0)
            e_t = og_p.tile([128, NK * F], f32, tag="e_t")
            nc.scalar.activation(e_t[:], mn[:], Act.Exp)
            o_t = og_p.tile([128, NK * F], f32, tag="o_t")
            nc.vector.scalar_tensor_tensor(
                o_t[:], e_t[:], 1.0, rt2[:], Alu.subtract, Alu.add
            )
            for k in range(NK):
                eng = nc.sync if k % 2 == 0 else nc.scalar
                eng.dma_start(
                    out_e[k * 128 : (k + 1) * 128, :], o_t[:, k * F : (k + 1) * F]
                )

    nc.compile()
    return nc


def _prepare_in_maps(x, adj, W, a):
    x2 = np.ascontiguousarray(x.reshape(BS, N).astype(np.float32))
    adj = np.asarray(adj, np.float32)
    W = np.asarray(W, np.float32)
    a = np.asarray(a, np.float32)
    c1 = float(np.float32(W[0] @ a[:F, 0]))
    c2 = float(np.float32(W[0] @ a[F:, 0]))

    xm = x2.mean(axis=1, dtype=np.float64).astype(np.float32)
    in_maps = []
    for k in range(NCORES):
        rb, bh = k % NRB, k // NRB
        i0 = rb * RB
        b0 = bh * BH
        x2h = x2[b0 : b0 + BH]
        adjT = np.ascontiguousarray(adj[i0 : i0 + RB, :].T)
        in_maps.append(
            {
                "adjT_b": adjT.astype(ml_dtypes.bfloat16),
                "xc1b": np.ascontiguousarray(np.float32(c1) * x2h[:, i0 : i0 + RB]),
                "xt": np.ascontiguousarray(
                    x2h.reshape(BH, NCHUNK, 128).transpose(0, 2, 1)
                ),
                "xmean": np.ascontiguousarray(
                    np.tile(xm[b0 : b0 + BH], NH).reshape(1, NK)
                ),
                "wmat": W,
            }
        )
    return in_maps, c1, c2


def kernel_with_results(x, adj, ext_input, side_input, W, a, trace=False):
    from concourse.bass_utils import run_bass_kernel_spmd

    in_maps, c1, c2 = _prepare_in_maps(x, adj, W, a)
    nc = _build(c1, c2)
    import time as _time
    res = None
    for attempt in range(3):
        try:
            res = run_bass_kernel_spmd(
                nc, in_maps, core_ids=list(range(NCORES)), trace=trace
            )
            break
        except Exception:
            if attempt == 2:
                raise
            _time.sleep(2.0)
    out = np.empty((BS, N, F), np.float32)
    for k in range(NCORES):
        rb, bh = k % NRB, k // NRB
        i0 = rb * RB
        b0 = bh * BH
        out[b0 : b0 + BH, i0 : i0 + RB, :] = res.results[k]["out"].reshape(BH, RB, F)
    return out, res


def kernel(**inputs):
    out, _ = kernel_with_results(
        inputs["x"], inputs["adj"], inputs.get("ext_input"),
        inputs.get("side_input"), inputs["W"], inputs["a"],
    )
    return out


# revision 27
# speedup vs baseline: 1.1925x; 1.1925x over previous
"""Trainium2 Bass kernel for the GAT-style message-passing layer.

Math reduction (exact w.r.t. the reference's masking semantics):
  h = x @ W is rank-1, so with c1 = W@a1, c2 = W@a2:
    e[b,i,j] = leakyrelu(c1*x_bi + c2*x_bj)
  After adjacency AND positivity masking, the softmax rows reduce to
    att[b,i,j] = m_ij * w_bj / D_bi,   w_bj = exp(c2*x_bj),
    D_bi = sum_j m_ij*w_bj,            m_ij = (adj_ij>0) & (c1*x_bi+c2*x_bj>0)
  (the exp(c1*x_bi) row factor cancels).  Then
    out[b,i,:] = ELU(s_bi * W),  s_bi = P_bi / D_bi,  P_bi = sum_j m_ij*w_bj*x_bj
  Fully-masked rows (D_bi == 0) fall back to the uniform softmax:
    s_bi = mean_j x_bj.

Sharding (8 cores): 4 row-blocks of the N dimension x 2 batch halves.
Each core owns 512 attention rows for 8 batches; it reads only its 4MB
row-slice of adj (transposed on host so j lands on partitions).

Per core the N^2 work is one fused mask op per [128 j x 512 i] chunk:
    R[j,i] = (c1*x_i  is_gt  -c2*x_j) * adjT[j,i]        (exact 0/1, bf16)
split across DVE (scalar_tensor_tensor) and ACT (saturated Sigmoid step,
with the adjacency multiply on Pool or DVE).  bf16 TensorE matmuls with the
chunk mask as the stationary and [w_j, w_j*x_j] as the moving operand reduce
each chunk into per-row (D, P) PSUM columns.  The epilogue computes
s = P/D with the degenerate-row blend and applies ELU(s*W) elementwise.
"""

import sys

import numpy as np

sys.path.insert(0, "/opt/trn_rl_repo")

import ml_dtypes  # noqa: E402

BS = 16
N = 2048
F = 40
NCORES = 8
NRB = 4                   # row blocks
NBH = 2                   # batch halves
RB = N // NRB             # 512 attention rows per core
BH = BS // NBH            # 8 batches per core
NCHUNK = N // 128         # 16 j-chunks
NH = RB // 128            # 4 stationary halves per chunk
BIGF = 1.0e30             # saturation scale for the ACT step path
ROWS_PER_CORE = BH * RB   # 4096 output rows per core
NK = ROWS_PER_CORE // 128 # 32 output chunks

# mask-chunk engine assignment by j-chunk index c (16 entries):
#   'd'  -> DVE scalar_tensor_tensor (one fused op)
#   'av' -> ACT saturated-Sigmoid step + DVE bf16 tensor_tensor multiply
#   'ap' -> ACT saturated-Sigmoid step + Pool tensor_tensor multiply
IMPL = ["d", "av", "ap", "av", "d", "ap", "av", "d", "ap", "av", "d", "ap", "av", "d", "ap", "av"]
USE_SIGN_PATH = False     # fallback: exact Sign+Relu ACT step (2 ACT passes)


def _build(c1: float, c2: float):
    import concourse.bass as bass  # noqa: F401
    import concourse.tile as tile
    from concourse import bacc, mybir

    f32 = mybir.dt.float32
    f16 = mybir.dt.float16
    bf16 = mybir.dt.bfloat16
    Alu = mybir.AluOpType
    Act = mybir.ActivationFunctionType

    nc = bacc.Bacc("TRN2", target_bir_lowering=False, debug=False)

    adjT_b = nc.declare_dram_parameter("adjT_b", [N, RB], bf16, isOutput=False)
    xc1b = nc.declare_dram_parameter("xc1b", [BH, RB], f16, isOutput=False)
    xt = nc.declare_dram_parameter("xt", [BH, 128, NCHUNK], f32, isOutput=False)
    xmean = nc.declare_dram_parameter("xmean", [1, NK], f32, isOutput=False)
    wmat = nc.declare_dram_parameter("wmat", [1, F], f32, isOutput=False)
    out_e = nc.declare_dram_parameter("out", [ROWS_PER_CORE, F], f32, isOutput=True)

    with tile.TileContext(nc) as tc:
        with (
            tc.tile_pool(name="const", bufs=1) as const,
            tc.tile_pool(name="wtmp", bufs=2) as wtmp_p,
            tc.tile_pool(name="xrep", bufs=3) as xrep_p,
            tc.tile_pool(name="gt", bufs=4) as gt_p,
            tc.tile_pool(name="rt", bufs=6) as rt_p,
            tc.tile_pool(name="acc", bufs=1, space="PSUM") as acc_p,
            tc.tile_pool(name="dp", bufs=1) as dp_p,
            tc.tile_pool(name="ep", bufs=1) as ep_p,
            tc.tile_pool(name="og", bufs=1) as og_p,
        ):
            # ---- constants / prologue -------------------------------------
            xt_t = const.tile([128, BH * NCHUNK], f32)  # col b*16+c = x[b, chunk c]
            nc.scalar.dma_start(xt_t[:], xt.rearrange("b p c -> p b c"))

            wrep = const.tile([128, F], f32)
            nc.scalar.dma_start(wrep[:], wmat[0:1, :].broadcast_to([128, F]))

            AGRP = 4  # adjacency chunk-group tiles (finer DMA/dep granularity)
            GC = NCHUNK // AGRP
            a_grp = []
            for g in range(AGRP):
                a_g = const.tile([128, GC * RB], bf16, tag=f"ag{g}")
                nc.sync.dma_start(
                    a_g[:],
                    adjT_b[g * GC * 128 : (g + 1) * GC * 128, :].rearrange(
                        "(c p) i -> p c i", p=128
                    ),
                )
                a_grp.append(a_g)

            # nv = -c2*x (stt threshold), bact = BIGF*c2*x (ACT step bias)
            nv_t = const.tile([128, BH * NCHUNK], f32)
            nc.gpsimd.tensor_scalar_mul(nv_t[:], xt_t[:], -c2)
            bact_t = const.tile([128, BH * NCHUNK], f32)
            nc.gpsimd.tensor_scalar_mul(bact_t[:], xt_t[:], BIGF * c2)

            # wy: interleaved [w_j, w_j*x_j] bf16 columns, 2 per (b, chunk)
            wy = const.tile([128, BH * 2 * NCHUNK], bf16)
            last_exp = None
            for b in range(BH):
                xb = xt_t[:, b * NCHUNK : (b + 1) * NCHUNK]
                w_f = wtmp_p.tile([128, NCHUNK], f32, tag="w_f")
                last_exp = nc.scalar.activation(w_f[:], xb, Act.Exp, bias=0.0, scale=c2)
                y_f = wtmp_p.tile([128, NCHUNK], f32, tag="y_f")
                nc.vector.tensor_mul(y_f[:], w_f[:], xb)
                base = b * 2 * NCHUNK
                nc.vector.tensor_copy(wy[:, base : base + 2 * NCHUNK : 2], w_f[:])
                nc.vector.tensor_copy(wy[:, base + 1 : base + 2 * NCHUNK : 2], y_f[:])

            # ---- main loop: masks + (D, P) reductions ---------------------
            # accs[h][:, 2b:2b+2] accumulates (D, P) for stationary half h of
            # batch b; one PSUM bank per half so accumulation groups never
            # share a bank's zero region.
            acc_0 = acc_p.tile([128, 2 * BH], f32, tag="acc0")
            acc_1 = acc_p.tile([128, 2 * BH], f32, tag="acc1")
            acc_2 = acc_p.tile([128, 2 * BH], f32, tag="acc2")
            acc_3 = acc_p.tile([128, 2 * BH], f32, tag="acc3")
            accs = [acc_0, acc_1, acc_2, acc_3]
            for b in range(BH):
                xr = xrep_p.tile([128, RB], f16)
                nc.scalar.dma_start(xr[:], xc1b[b : b + 1, :].broadcast_to([128, RB]))
                for c in range(NCHUNK):
                    col = b * NCHUNK + c
                    a_chunk_b = a_grp[c // 4][:, (c % 4) * RB : (c % 4 + 1) * RB]
                    r = rt_p.tile([128, RB], bf16)
                    kind = IMPL[c]
                    if kind.startswith("a"):
                        g = gt_p.tile([128, RB], bf16)
                        if USE_SIGN_PATH:
                            g2 = gt_p.tile([128, RB], f32, tag="g2")
                            nc.scalar.activation(
                                g2[:], xr[:], Act.Sign,
                                bias=bact_t[:, col : col + 1], scale=BIGF,
                            )
                            nc.scalar.activation(g[:], g2[:], Act.Relu)
                        else:
                            sig = nc.scalar.activation(
                                g[:], xr[:], Act.Sigmoid,
                                bias=bact_t[:, col : col + 1], scale=BIGF,
                            )
                            if last_exp is not None:
                                from concourse.tile import add_dep_helper
                                add_dep_helper(
                                    sig.ins, last_exp.ins,
                                    reason="act table: exps before sigmoids",
                                )
                                last_exp = None
                        eng = nc.vector if kind == "av" else nc.gpsimd
                        eng.tensor_mul(r[:], g[:], a_chunk_b)
                    else:
                        nc.vector.scalar_tensor_tensor(
                            r[:], xr[:], nv_t[:, col : col + 1], a_chunk_b,
                            Alu.is_gt, Alu.mult,
                        )
                    wy_c = wy[:, b * 2 * NCHUNK + 2 * c : b * 2 * NCHUNK + 2 * c + 2]
                    for h in range(NH):
                        nc.tensor.matmul(
                            accs[h][:, 2 * b : 2 * b + 2],
                            r[:, h * 128 : (h + 1) * 128],
                            wy_c,
                            start=(c == 0), stop=(c == NCHUNK - 1),
                        )

            # ---- epilogue: s = P/D with uniform fallback ------------------
            # epilogue column t = h*BH + b (half-major)
            dp_sb = dp_p.tile([128, 2 * NK], f32)
            for h in range(NH):
                nc.vector.tensor_copy(
                    dp_sb[:, h * 2 * BH : (h + 1) * 2 * BH], accs[h][:]
                )
            d_v = dp_sb[:, 0 : 2 * NK : 2]   # [128, 32]
            p_v = dp_sb[:, 1 : 2 * NK : 2]   # [128, 32]
            xmean_r = const.tile([128, NK], f32)
            nc.sync.dma_start(xmean_r[:], xmean[0:1, :].broadcast_to([128, NK]))
            dmax = ep_p.tile([128, NK], f32)
            nc.vector.tensor_scalar_max(dmax[:], d_v, 1e-30)
            rec = ep_p.tile([128, NK], f32)
            nc.vector.reciprocal(rec[:], dmax[:])
            s0 = ep_p.tile([128, NK], f32)
            nc.vector.tensor_mul(s0[:], p_v, rec[:])
            flag = ep_p.tile([128, NK], f32)
            nc.vector.tensor_scalar(flag[:], d_v, 0.0, None, Alu.is_gt)
            t1 = ep_p.tile([128, NK], f32)
            nc.vector.tensor_sub(t1[:], s0[:], xmean_r[:])
            t2 = ep_p.tile([128, NK], f32)
            nc.vector.tensor_mul(t2[:], t1[:], flag[:])
            s = ep_p.tile([128, NK], f32)
            nc.vector.tensor_add(s[:], t2[:], xmean_r[:])

            # ---- output: ELU(s * W); chunk k covers out rows [128k, 128k+128)
            t_all = og_p.tile([128, NK * F], f32, tag="t_all")
            for k in range(NK):
                col = (k % NH) * BH + (k // NH)  # half-major epilogue column
                dst = t_all[:, k * F : (k + 1) * F]
                if k % 2 == 0:
                    nc.vector.tensor_scalar_mul(dst, wrep[:], s[:, col : col + 1])
                else:
                    nc.scalar.activation(
                        dst, wrep[:], Act.Copy, bias=0.0, scale=s[:, col : col + 1]
                    )
            mn = og_p.tile([128, NK * F], f32, tag="mn")
            nc.vector.tensor_scalar_min(mn[:], t_all[:], 0.0)
            rt2 = og_p.tile([128, NK * F], f32, tag="rt2")
            nc.vector.tensor_scalar_max(rt2[:], t_all[:], 0.0)
            e_t = og_p.tile([128, NK * F], f32, tag="e_t")
            nc.scalar.activation(e_t[:], mn[:], Act.Exp)
            o_t = og_p.tile([128, NK * F], f32, tag="o_t")
            nc.vector.scalar_tensor_tensor(
                o_t[:], e_t[:], 1.0, rt2[:], Alu.subtract, Alu.add
            )
            for k in range(NK):
                eng = nc.sync if k % 2 == 0 else nc.scalar
                eng.dma_start(
                    out_e[k * 128 : (k + 1) * 128, :], o_t[:, k * F : (k + 1) * F]
                )

    nc.compile()
    return nc


def _prepare_in_maps(x, adj, W, a):
    x2 = np.ascontiguousarray(x.reshape(BS, N).astype(np.float32))
    adj = np.asarray(adj, np.float32)
    W = np.asarray(W, np.float32)
    a = np.asarray(a, np.float32)
    c1 = float(np.float32(W[0] @ a[:F, 0]))
    c2 = float(np.float32(W[0] @ a[F:, 0]))

    xm = x2.mean(axis=1, dtype=np.float64).astype(np.float32)
    in_maps = []
    for k in range(NCORES):
        rb, bh = k % NRB, k // NRB
        i0 = rb * RB
        b0 = bh * BH
        x2h = x2[b0 : b0 + BH]
        adjT = np.ascontiguousarray(adj[i0 : i0 + RB, :].T)
        in_maps.append(
            {
                "adjT_b": adjT.astype(ml_dtypes.bfloat16),
                "xc1b": np.ascontiguousarray(
                    (np.float32(c1) * x2h[:, i0 : i0 + RB]).astype(np.float16)
                ),
                "xt": np.ascontiguousarray(
                    x2h.reshape(BH, NCHUNK, 128).transpose(0, 2, 1)
                ),
                "xmean": np.ascontiguousarray(
                    np.tile(xm[b0 : b0 + BH], NH).reshape(1, NK)
                ),
                "wmat": W,
            }
        )
    return in_maps, c1, c2


def kernel_with_results(x, adj, ext_input, side_input, W, a, trace=False):
    from concourse.bass_utils import run_bass_kernel_spmd

    in_maps, c1, c2 = _prepare_in_maps(x, adj, W, a)
    nc = _build(c1, c2)
    import time as _time
    res = None
    for attempt in range(3):
        try:
            res = run_bass_kernel_spmd(
                nc, in_maps, core_ids=list(range(NCORES)), trace=trace
            )
            break
        except Exception:
            if attempt == 2:
                raise
            _time.sleep(2.0)
    out = np.empty((BS, N, F), np.float32)
    for k in range(NCORES):
        rb, bh = k % NRB, k // NRB
        i0 = rb * RB
        b0 = bh * BH
        out[b0 : b0 + BH, i0 : i0 + RB, :] = res.results[k]["out"].reshape(BH, RB, F)
    return out, res


def kernel(**inputs):
    out, _ = kernel_with_results(
        inputs["x"], inputs["adj"], inputs.get("ext_input"),
        inputs.get("side_input"), inputs["W"], inputs["a"],
    )
    return out
